# revision 17
# baseline (speedup 1.0000x reference)
# Distributed GIN (3-layer) + per-graph softmax on 8 TRN2 NeuronCores.
#
# Sharding: nodes split into 8 contiguous ranges of 6250; edges partitioned by
# destination core so segment_sum is local (computed as one-hot matmuls on the
# TensorEngine); per layer the node features (x-space: the BN/relu output xn,
# NOT y = x @ W1 — W1 is applied post-aggregation on every layer, exploiting
# linearity) are all-gathered into replicated DRAM tables that each core
# gathers its edge sources from via dma_gather.  The table is split into two
# halves (a/b by within-shard offset) so each half's AllGather overlaps
# compute and the int16 gather indices stay in range.
#
# Each gather layer runs in TWO PHASES: an a-phase that processes every
# dst-tile's a-stream edges (self term + a matmuls) into an SBUF f32 partial
# strip, then a b-phase that adds the b-stream matmuls and runs the MLP
# epilogue.  This removes the b-half AllGather from the critical path (it
# flies during the a-phase) and keeps the GpSimd queue free of head-of-line
# blocking (a-phase never enqueues a gather that waits on the b collective).
#
# Layer 0 needs no gather: the host materializes x[src] directly.  The
# per-graph softmax partial sums are accumulated inside layer 2's b-phase
# epilogues (2 tiny matmuls per tile) so the tail is just a [128,2]-wide
# AllGather + the final scale pass.
import numpy as np
import ml_dtypes

N = 50000
E = 800000
G = 256
DIMS = [128, 128, 64, 32]
BN_EPS = 1e-5
TEMP = 5.0

NCORES = 8
NLOC = N // NCORES            # 6250
NTILE = 49                    # node tiles per core
NPAD = NTILE * 128            # 6272
ASPL = 3072                   # within-shard offset split for a/b table halves
NA = ASPL * NCORES            # 24576 rows in table a
BROWS = NPAD - ASPL           # 3200 rows per core in table b (tile-aligned, padded)
NB = BROWS * NCORES           # 25600 rows in table b
CH = 2                        # dst-tiles per gather chunk
LEAD = 10                     # gather chunks issued ahead of consumption
OHLEAD = 2                    # one-hot build chunks ahead of consumption
CCA_AT = 23                   # issue half-a AllGather once dst-tiles 0..23 done

MLP_IN = [128, 128, 64]
MLP_M = [128, 64, 32]

BF16 = ml_dtypes.bfloat16

_CACHE = {}


def _chunks():
    out = []
    d = 0
    while d < NTILE:
        out.append((d, min(CH, NTILE - d)))
        d += CH
    return out


def _pack_stream(vals, drel, budgets, chunks):
    """vals/drel: per-dst-tile lists.  Packs pair-row gather indices plus
    TWO pre-masked drel arrays (even/odd source parity; -1 masks the edge out
    of the other parity's one-hot).  Returns (idx[128, .], drel_even[128, .],
    drel_odd[128, .], tile offsets)."""
    offs = np.zeros(NTILE + 1, dtype=np.int64)
    np.cumsum(budgets, out=offs[1:])
    tot = int(offs[-1])
    iv = np.zeros(tot * 128, dtype=np.int64)
    dev = np.full(tot * 128, -1.0, dtype=np.float32)
    dov = np.full(tot * 128, -1.0, dtype=np.float32)
    for d in range(NTILE):
        n = len(vals[d])
        base = int(offs[d]) * 128
        iv[base:base + n] = vals[d] // 2
        par = (vals[d] % 2).astype(np.float32)
        dev[base:base + n] = np.where(par == 0, drel[d], -1.0)
        dov[base:base + n] = np.where(par == 1, drel[d], -1.0)
    cols = []
    for d0, csz in chunks:
        v = iv[offs[d0] * 128:offs[d0 + csz] * 128]
        cols.append(v.reshape(-1, 16).T)
    w = np.tile(np.concatenate(cols, axis=1).astype(np.int16), (8, 1))
    de_t = dev.reshape(tot, 128).T                     # [slot, tile]
    do_t = dov.reshape(tot, 128).T
    # pre-built one-hots, interleaved even/odd per budget tile:
    # [slot, tile, 0:128] = (drel_even == col), [slot, tile, 128:256] = odd
    ar = np.arange(128, dtype=np.float32)
    ohp = np.empty((128, tot, 256), dtype=BF16)
    ohp[:, :, :128] = (de_t[:, :, None] == ar)
    ohp[:, :, 128:] = (do_t[:, :, None] == ar)
    return w, ohp.reshape(128, tot * 256), offs


def _preprocess(x, edge_index, batch):
    src = np.asarray(edge_index[0], dtype=np.int64)
    dst = np.asarray(edge_index[1], dtype=np.int64)
    batch = np.asarray(batch, dtype=np.int64)
    x = np.asarray(x, dtype=np.float32)
    chunks = _chunks()

    owner = src // NLOC
    off = src % NLOC
    s_ab = (off >= ASPL).astype(np.int64)          # 0 = a, 1 = b
    row = np.where(s_ab == 0, owner * ASPL + off, owner * BROWS + off - ASPL)

    core = dst // NLOC
    dtile = (dst % NLOC) // 128
    key = (core * NTILE + dtile) * 2 + s_ab
    order = np.argsort(key, kind="stable")
    srow = row[order]
    sdst = dst[order]
    ssrc = src[order]
    counts = np.bincount(key, minlength=NCORES * NTILE * 2).reshape(NCORES, NTILE, 2)
    starts = np.zeros(NCORES * NTILE * 2 + 1, dtype=np.int64)
    np.cumsum(counts.reshape(-1), out=starts[1:])
    # per-(dst-tile, stream) tile budgets, shared across cores (max)
    B = np.ceil(counts.max(axis=0) / 128).astype(np.int64)      # [NTILE, 2]
    # layer-0 combined stream budgets
    ccounts = counts.sum(axis=2)                                 # [NCORES, NTILE]
    B0 = np.ceil(ccounts.max(axis=0) / 128).astype(np.int64)     # [NTILE]

    per_core = []
    for r in range(NCORES):
        vals = {0: [], 1: []}
        drels = {0: [], 1: []}
        xevals = []
        xdrels = []
        for d in range(NTILE):
            xs = []
            xd = []
            for s in (0, 1):
                k = (r * NTILE + d) * 2 + s
                a, b = starts[k], starts[k + 1]
                vals[s].append(srow[a:b])
                dr = (sdst[a:b] - (r * NLOC + d * 128)).astype(np.float32)
                drels[s].append(dr)
                xs.append(ssrc[a:b])
                xd.append(dr)
            xevals.append(np.concatenate(xs))
            xdrels.append(np.concatenate(xd))
        idx_a, ohp_a, offa = _pack_stream(vals[0], drels[0], B[:, 0], chunks)
        idx_b, ohp_b, offb = _pack_stream(vals[1], drels[1], B[:, 1], chunks)

        # layer-0 host-materialized edge stream (x[src], edge-major, pre-tiled)
        off0 = np.zeros(NTILE + 1, dtype=np.int64)
        np.cumsum(B0, out=off0[1:])
        tot0 = int(off0[-1])
        xe = np.zeros((tot0 * 128, 128), dtype=np.float32)
        drc = np.full(tot0 * 128, -1.0, dtype=np.float32)
        for d in range(NTILE):
            n = len(xevals[d])
            base = int(off0[d]) * 128
            xe[base:base + n] = x[xevals[d]]
            drc[base:base + n] = xdrels[d]
        xe_t = xe.reshape(tot0, 128, 128).transpose(1, 0, 2).reshape(128, tot0 * 128).astype(BF16)
        drc_t = drc.reshape(tot0, 128).T.astype(BF16)

        bl = batch[r * NLOC:(r + 1) * NLOC].astype(np.float32)
        bpad = np.concatenate([bl, np.full(NPAD - NLOC, -1.0, np.float32)])
        xp = np.zeros((NPAD, 128), np.float32)
        xp[:NLOC] = x[r * NLOC:(r + 1) * NLOC]
        per_core.append(dict(
            x_bf=xp.astype(BF16), x_edges=xe_t, drel_c=drc_t,
            idx_a=idx_a, idx_b=idx_b, ohp_a=ohp_a, ohp_b=ohp_b,
            brow=np.tile(bpad, (128, 1)).astype(BF16),
            batchT=bpad.reshape(NTILE, 128).T.astype(BF16),
        ))
    shape_key = (tuple(B[:, 0]), tuple(B[:, 1]), tuple(B0))
    return per_core, shape_key


def _weights(inputs):
    w = {}
    for l in range(3):
        w[f"w1_{l}"] = np.ascontiguousarray(np.asarray(inputs[f"W1_{l}"], np.float32)).astype(BF16)
        w[f"w2_{l}"] = np.ascontiguousarray(np.asarray(inputs[f"W2_{l}"], np.float32)).astype(BF16)
    # layer-2 W1 stacked twice: folds the even/odd parity halves of the
    # aggregation PSUM (rows 0:63 / 64:127) during the W1 matmul
    w12 = np.asarray(inputs["W1_2"], np.float32)
    w["w1_2s"] = np.concatenate([w12, w12], axis=0).astype(BF16)   # [128, 32]
    fold = np.zeros((128, 64), np.float32)
    fold[np.arange(64), np.arange(64)] = 1.0
    fold[np.arange(64, 128), np.arange(64)] = 1.0
    w["fold64"] = fold.astype(BF16)
    hs = np.zeros((128, 128), np.float32)
    hs[:, 64:] = 1.0
    w["halfsel"] = hs.astype(BF16)                                  # row: 0|1 halves
    w["wlin"] = (np.asarray(inputs["W_lin"], np.float32) / TEMP).astype(BF16)
    vec = np.zeros((128, 11), np.float32)
    for l in range(3):
        m = MLP_M[l]
        g = np.asarray(inputs[f"gamma_{l}"], np.float32)
        be = np.asarray(inputs[f"beta_{l}"], np.float32)
        mu = np.asarray(inputs[f"mean_{l}"], np.float32)
        va = np.asarray(inputs[f"var_{l}"], np.float32)
        b1 = np.asarray(inputs[f"b1_{l}"], np.float32)
        b2 = np.asarray(inputs[f"b2_{l}"], np.float32)
        scale = g / np.sqrt(va + BN_EPS)
        shift = be - mu * scale + b2 * scale
        vec[:m, 3 * l + 0] = b1
        vec[:m, 3 * l + 1] = scale
        vec[:m, 3 * l + 2] = shift
    vec[:, 9] = np.arange(128, dtype=np.float32)
    vec[:, 10] = np.arange(128, dtype=np.float32) + 128.0
    w["vec"] = vec
    w["blin_t"] = float(np.asarray(inputs["b_lin"], np.float32).reshape(-1)[0]) / TEMP
    ar = np.arange(128, dtype=np.float32)
    w["iota_e"] = np.tile(ar, (128, 1)).astype(BF16)
    w["iota_g0"] = np.tile(ar, (128, 1)).astype(BF16)
    w["iota_g1"] = (np.tile(ar, (128, 1)) + 128.0).astype(BF16)
    w["ident_b"] = np.eye(128, dtype=np.float32).astype(BF16)
    return w


def _build(shape_key, blin_t):
    import concourse.bacc as bacc
    import concourse.tile as tile
    from concourse import mybir

    f32 = mybir.dt.float32
    bf16 = mybir.dt.bfloat16
    i16 = mybir.dt.int16
    RELU = mybir.ActivationFunctionType.Relu
    IDENT = mybir.ActivationFunctionType.Identity
    EXP = mybir.ActivationFunctionType.Exp
    EQ = mybir.AluOpType.is_equal
    ADD = mybir.AluOpType.add

    Ba = np.array(shape_key[0], dtype=np.int64)
    Bb = np.array(shape_key[1], dtype=np.int64)
    B0 = np.array(shape_key[2], dtype=np.int64)
    offa = np.zeros(NTILE + 1, np.int64); np.cumsum(Ba, out=offa[1:])
    offb = np.zeros(NTILE + 1, np.int64); np.cumsum(Bb, out=offb[1:])
    off0 = np.zeros(NTILE + 1, np.int64); np.cumsum(B0, out=off0[1:])
    TOTA, TOTB, TOT0 = int(offa[-1]), int(offb[-1]), int(off0[-1])
    chunks = _chunks()
    maxnt = {"a": max(int(offa[d0 + c] - offa[d0]) for d0, c in chunks),
             "b": max(int(offb[d0 + c] - offb[d0]) for d0, c in chunks),
             "c": max(int(off0[d0 + c] - off0[d0]) for d0, c in chunks)}
    gmax = max(maxnt["a"], maxnt["b"])

    nc = bacc.Bacc("TRN2", target_bir_lowering=False, debug=False,
                   num_devices=NCORES, num_swdge_queues=4,
                   dynamic_dma_scratch_size=32768)

    x_in = nc.dram_tensor("x_bf", [NPAD, 128], bf16, kind="ExternalInput")
    xe_in = nc.dram_tensor("x_edges", [128, TOT0 * 128], bf16, kind="ExternalInput")
    drelc_in = nc.dram_tensor("drel_c", [128, TOT0], bf16, kind="ExternalInput")
    idx_in = {"a": nc.dram_tensor("idx_a", [128, TOTA * 8], i16, kind="ExternalInput"),
              "b": nc.dram_tensor("idx_b", [128, TOTB * 8], i16, kind="ExternalInput")}
    ohp_in = {"a": nc.dram_tensor("ohp_a", [128, TOTA * 256], bf16, kind="ExternalInput"),
              "b": nc.dram_tensor("ohp_b", [128, TOTB * 256], bf16, kind="ExternalInput")}

    brow_in = nc.dram_tensor("brow", [128, NPAD], bf16, kind="ExternalInput")
    batchT_in = nc.dram_tensor("batchT", [128, NTILE], bf16, kind="ExternalInput")
    w1_in = [nc.dram_tensor(f"w1_{l}", [MLP_IN[l], MLP_M[l]], bf16, kind="ExternalInput") for l in range(3)]
    w2_in = [nc.dram_tensor(f"w2_{l}", [MLP_M[l], MLP_M[l]], bf16, kind="ExternalInput") for l in range(3)]
    wlin_in = nc.dram_tensor("wlin", [32, 1], bf16, kind="ExternalInput")
    vec_in = nc.dram_tensor("vec", [128, 11], f32, kind="ExternalInput")
    iota_e_in = nc.dram_tensor("iota_e", [128, 128], bf16, kind="ExternalInput")
    iota_g0_in = nc.dram_tensor("iota_g0", [128, 128], bf16, kind="ExternalInput")
    iota_g1_in = nc.dram_tensor("iota_g1", [128, 128], bf16, kind="ExternalInput")
    ident_b_in = nc.dram_tensor("ident_b", [128, 128], bf16, kind="ExternalInput")
    out_dram = nc.dram_tensor("out", [NPAD, 1], f32, kind="ExternalOutput")

    rg = [list(range(NCORES))]

    with tile.TileContext(nc) as tc:
        with (
            tc.tile_pool(name="persist", bufs=1) as pp,
            tc.tile_pool(name="dram", bufs=1, space="DRAM") as dp,
            tc.tile_pool(name="work", bufs=3) as wp,
            tc.tile_pool(name="psA", bufs=2, space="PSUM") as psA,
            tc.tile_pool(name="psB", bufs=5, space="PSUM") as psB,
            tc.tile_pool(name="psG", bufs=2, space="PSUM") as psG,
            tc.tile_pool(name="gate", bufs=2) as ggp,
        ):
            idx_sb = {}
            for s in ("a", "b"):
                wdt = TOTA if s == "a" else TOTB
                idx_sb[s] = pp.tile([128, wdt * 8], i16, tag=f"idx_{s}", name=f"idx_{s}")
                nc.sync.dma_start(out=idx_sb[s][:], in_=idx_in[s][:])
            drel_c = pp.tile([128, TOT0], bf16, tag="drel_c", name="drel_c")
            nc.sync.dma_start(out=drel_c[:], in_=drelc_in[:])
            batchT = pp.tile([128, NTILE], bf16, tag="batchT", name="batchT")
            nc.sync.dma_start(out=batchT[:], in_=batchT_in[:])
            w1 = []
            w2 = []
            for l in range(3):
                t1_ = pp.tile([MLP_IN[l], MLP_M[l]], bf16, tag=f"w1_{l}", name=f"w1s_{l}")
                nc.sync.dma_start(out=t1_[:], in_=w1_in[l][:])
                w1.append(t1_)
                t2_ = pp.tile([MLP_M[l], MLP_M[l]], bf16, tag=f"w2_{l}", name=f"w2s_{l}")
                nc.sync.dma_start(out=t2_[:], in_=w2_in[l][:])
                w2.append(t2_)
            wlin = pp.tile([32, 1], bf16, tag="wlin", name="wlin")
            nc.sync.dma_start(out=wlin[:], in_=wlin_in[:])
            vec = pp.tile([128, 11], f32, tag="vec", name="vec")
            nc.sync.dma_start(out=vec[:], in_=vec_in[:])
            iota_e = pp.tile([128, 128], bf16, tag="iota_e", name="iota_e")
            nc.sync.dma_start(out=iota_e[:], in_=iota_e_in[:])
            iota_g0 = pp.tile([128, 128], bf16, tag="iota_g0", name="iota_g0")
            nc.sync.dma_start(out=iota_g0[:], in_=iota_g0_in[:])
            iota_g1 = pp.tile([128, 128], bf16, tag="iota_g1", name="iota_g1")
            nc.sync.dma_start(out=iota_g1[:], in_=iota_g1_in[:])
            ident_b = pp.tile([128, 128], bf16, tag="ident_b", name="ident_b")
            nc.sync.dma_start(out=ident_b[:], in_=ident_b_in[:])

            # node-major xn strips: self-edge source for layer l (strip[l%2]),
            # written during layer l-1 (doubles as DMA staging for the table).
            strip = [pp.tile([128, NPAD], bf16, tag=f"strip{i}", name=f"strip{i}") for i in range(2)]
            nc.sync.dma_start(
                out=strip[0][:].rearrange("p (i e) -> p i e", e=128),
                in_=x_in[:].rearrange("(i p) e -> p i e", p=128),
            )
            # f32 partial aggregation strip (a-phase output)
            partial = pp.tile([128, NPAD], f32, tag="partial", name="partial")
            e_strip = pp.tile([128, NTILE], f32, tag="e_strip", name="e_strip")
            e_b = pp.tile([128, NTILE], bf16, tag="e_b", name="e_b")
            out_strip = pp.tile([128, NTILE], f32, tag="out_strip", name="out_strip")
            r_str = pp.tile([128, NTILE], f32, tag="r_str", name="r_str")

            # static softmax one-hots (gsum side), built once up front
            sbt = pp.tile([128, NPAD], bf16, tag="sbt", name="sbt")
            sbt2 = pp.tile([128, NPAD], bf16, tag="sbt2", name="sbt2")
            for sb_h, iog in ((sbt, iota_g0), (sbt2, iota_g1)):
                nc.vector.tensor_tensor(
                    out=sb_h[:].rearrange("p (t e) -> p t e", e=128),
                    in0=batchT[:].rearrange("p (t o) -> p t o", o=1).to_broadcast([128, NTILE, 128]),
                    in1=iog[:].rearrange("p (o e) -> p o e", o=1).to_broadcast([128, NTILE, 128]),
                    op=EQ)

            yl_a = [None] + [dp.tile([ASPL, 64], bf16, tag=f"yla{l}", name=f"yla{l}") for l in (1, 2)]
            yl_b = [None] + [dp.tile([NPAD - ASPL, 64], bf16, tag=f"ylb{l}", name=f"ylb{l}") for l in (1, 2)]
            tf_a = [None] + [dp.tile([NA, 64], bf16, tag=f"tfa{l}", name=f"tfa{l}") for l in (1, 2)]
            tf_b = [None] + [dp.tile([NB, 64], bf16, tag=f"tfb{l}", name=f"tfb{l}") for l in (1, 2)]
            ag_in = dp.tile([128, 2], f32, tag="ag_in", name="ag_in")
            ag_out = dp.tile([NCORES * 128, 2], f32, tag="ag_out", name="ag_out")

            ATIL = ASPL // 128   # 24 tiles in the a-range

            def dma_rows(l, r0, nrow_t, src_sb):
                if r0 < ATIL:
                    seg = yl_a[l][r0 * 128:(r0 + nrow_t) * 128, :]
                else:
                    seg = yl_b[l][(r0 - ATIL) * 128:(r0 - ATIL + nrow_t) * 128, :]
                nc.sync.dma_start(
                    out=seg.rearrange("(i p) e -> p i e", p=128),
                    in_=src_sb.rearrange("p (i e) -> p i e", e=128)[:, :, 0:64],
                )

            def cc_half(l, half):
                if half == 0:
                    nc.gpsimd.collective_compute(
                        "AllGather", mybir.AluOpType.bypass, replica_groups=rg,
                        ins=[yl_a[l][:]], outs=[tf_a[l][:]])
                else:
                    nc.gpsimd.collective_compute(
                        "AllGather", mybir.AluOpType.bypass, replica_groups=rg,
                        ins=[yl_b[l][:]], outs=[tf_b[l][:]])

            def build_oh(dst_tile, drl_src, c0, nt):
                drl = drl_src[:, c0:c0 + nt]
                nc.vector.tensor_tensor(
                    out=dst_tile[:, :nt, :],
                    in0=drl.rearrange("p (t o) -> p t o", o=1).to_broadcast([128, nt, 128]),
                    in1=iota_e[:].rearrange("p (o e) -> p o e", o=1).to_broadcast([128, nt, 128]),
                    op=EQ)

            # ================= per-layer epilogue =========
            # l=0: t1 = x-space agg (128-wide); MLP; also builds the y-space
            #      strip for layer 1 (y1 = xn0 @ W1_1).
            # l=1: t1 = y-space agg, parity halves in rows 0:63 / 64:127;
            #      fold64 matmul folds them, then the MLP (no W1 — y-space).
            # l=2: t1 = x-space agg, parity halves; stacked W1_2 folds + maps.
            def epilogue(l, t, t1, pos=0):
                m = MLP_M[l]
                last = l == 2
                h = wp.tile([128, 128], bf16, tag="h", name="h")
                if l == 1:
                    # y-space aggregation: W1 already inside the table values
                    nc.scalar.activation(h[:m, :], t1[:m, :], RELU,
                                         bias=vec[:m, 3 * l:3 * l + 1], scale=1.0)
                else:
                    ps_i = psB.tile([128, 128], f32, tag="ps_tmp", name="ps_i")
                    din = MLP_IN[l]
                    nc.tensor.matmul(out=ps_i[:m, :], lhsT=w1[l][:], rhs=t1[:din, :],
                                     start=True, stop=True)
                    nc.scalar.activation(h[:m, :], ps_i[:m, :], RELU,
                                         bias=vec[:m, 3 * l:3 * l + 1], scale=1.0)
                ps_z = psB.tile([128, 128], f32, tag="ps_tmp", name="ps_z")
                nc.tensor.matmul(out=ps_z[:m, :], lhsT=w2[l][:], rhs=h[:m, :],
                                 start=True, stop=True)
                xn = wp.tile([128, 128], bf16, tag="xn", name="xn")
                nc.scalar.activation(xn[:m, :], ps_z[:m, :],
                                     IDENT if last else RELU,
                                     bias=vec[:m, 3 * l + 2:3 * l + 3],
                                     scale=vec[:m, 3 * l + 1:3 * l + 2])
                if l == 0:
                    # y-space strip for layer 1: y1 = xn @ W1_1 (64-wide)
                    ps_y = psB.tile([128, 128], f32, tag="ps_tmp", name="ps_y")
                    nc.tensor.matmul(out=ps_y[:64, :], lhsT=w1[1][:], rhs=xn[:, :],
                                     start=True, stop=True)
                    ty = wp.tile([128, 128], bf16, tag="ty", name="ty")
                    nc.scalar.copy(out=ty[:64, :], in_=ps_y[:64, :])
                    ps_t = psB.tile([128, 128], bf16, tag="ps_tmp", name="ps_t2")
                    nc.tensor.transpose(out=ps_t[:], in_=ty[:, :], identity=ident_b[:])
                    nc.scalar.copy(out=strip[1][:, t * 128:t * 128 + 64], in_=ps_t[:, 0:64])
                elif l == 1:
                    # x-space strip for layer 2: xn1 (64-wide)
                    ps_t = psB.tile([128, 128], bf16, tag="ps_tmp", name="ps_t2")
                    nc.tensor.transpose(out=ps_t[:], in_=xn[:, :], identity=ident_b[:])
                    nc.scalar.copy(out=strip[0][:, t * 128:t * 128 + 64], in_=ps_t[:, 0:64])
                else:
                    ps_lg = psB.tile([128, 1], f32, tag="ps_tmp", name="ps_lg")
                    nc.tensor.matmul(out=ps_lg[:], lhsT=xn[:m, :], rhs=wlin[:],
                                     start=True, stop=True)
                    nc.scalar.activation(e_strip[:, t:t + 1], ps_lg[:], EXP,
                                         bias=blin_t, scale=1.0)
                    nc.vector.tensor_copy(out=e_b[:, t:t + 1], in_=e_strip[:, t:t + 1])
                    # accumulate per-graph softmax partial sums (separate PSUM
                    # tiles: a shared tile would let one group's start=True
                    # clear the other's has_written bits)
                    nc.tensor.matmul(out=pg0[:], lhsT=sbt[:, t * 128:(t + 1) * 128],
                                     rhs=e_b[:, t:t + 1],
                                     start=(pos == 0), stop=(pos == NTILE - 1))
                    nc.tensor.matmul(out=pg1[:], lhsT=sbt2[:, t * 128:(t + 1) * 128],
                                     rhs=e_b[:, t:t + 1],
                                     start=(pos == 0), stop=(pos == NTILE - 1))

            pg0 = psG.tile([128, 1], f32, tag="pg", name="pg0")
            pg1 = psG.tile([128, 1], f32, tag="pg", name="pg1")

            # ---------- layer 0 (host-materialized edge stream, single pass) --
            # Epilogue chunk order: tiles 24..48 first, then 0..23.  The
            # b-half table (rows 24..48) all-gathers mid-layer — during L0's
            # gather-free window / with a full pipeline in layers 1-2 — and
            # the a-half fires at layer end with a full phase of slack.
            ECH = chunks[12:] + chunks[:12]
            with (
                tc.tile_pool(name="gx", bufs=4) as gpx,
                tc.tile_pool(name="gox", bufs=3) as gox,
            ):
                for d0, csz in ECH:
                    nt = int(off0[d0 + csz] - off0[d0])
                    xe = gpx.tile([128, maxnt["c"], 128], bf16, tag="g_x", name="xe")
                    nc.sync.dma_start(
                        out=xe[:, :nt, :].rearrange("p t e -> p (t e)"),
                        in_=xe_in[:, off0[d0] * 128:off0[d0 + csz] * 128])
                    oh0 = gox.tile([128, maxnt["c"], 128], bf16, tag="oh_x", name="oh0")
                    build_oh(oh0, drel_c, int(off0[d0]), nt)
                    for i in range(csz):
                        t = d0 + i
                        ps_agg = psA.tile([128, 128], f32, tag="ps_agg", name="ps_agg")
                        nb_ = int(B0[t])
                        nc.tensor.matmul(
                            out=ps_agg[:], lhsT=strip[0][:, t * 128:(t + 1) * 128],
                            rhs=ident_b[:], start=True, stop=(nb_ == 0))
                        base = int(off0[t] - off0[d0])
                        for j in range(nb_):
                            nc.tensor.matmul(
                                out=ps_agg[:], lhsT=xe[:, base + j, :], rhs=oh0[:, base + j, :],
                                start=False, stop=(j == nb_ - 1))
                        t1 = wp.tile([128, 128], bf16, tag="t1", name="t1")
                        nc.scalar.copy(out=t1[:], in_=ps_agg[:])
                        epilogue(0, t, t1)
                    dma_rows(1, d0, csz, strip[1][:, d0 * 128:(d0 + csz) * 128])
                    if d0 == NTILE - 1:
                        cc_half(1, 1)
                cc_half(1, 0)
            # (gate for cc(1,0) is issued right after the helper definition)
            gate_rr = [0]

            def cc_gate(tfx, idx_src):
                gd = ggp.tile([128, 1, 128], bf16, tag="g_gate", name="g_gate")
                nc.gpsimd.dma_gather(
                    gd[:, :, :], tfx[:].rearrange("(r two) e -> r (two e)", two=2),
                    idx_sb[idx_src][:, 0:8],
                    num_idxs=128, num_idxs_reg=128, elem_size=128,
                    single_packet=False, queue_num=gate_rr[0] % 4)
                gate_rr[0] += 1

            # ---------- layers 1, 2 (table gathers, two phases) ----------
            with (
                tc.tile_pool(name="ga", bufs=LEAD + 1) as gpa,
                tc.tile_pool(name="goa", bufs=OHLEAD + 2) as goa,
            ):
                for l in (1, 2):
                    last = l == 2
                    nch = len(chunks)
                    qrr = [l]

                    def issue_g(ci, s, store):
                        d0, csz = chunks[ci]
                        offs = offa if s == "a" else offb
                        tfx = tf_a[l] if s == "a" else tf_b[l]
                        nt = int(offs[d0 + csz] - offs[d0])
                        g = gpa.tile([128, gmax, 128], bf16, tag="g_s", name="g_s")
                        nc.gpsimd.dma_gather(
                            g[:, :nt, :],
                            tfx[:].rearrange("(r two) e -> r (two e)", two=2),
                            idx_sb[s][:, int(offs[d0]) * 8:int(offs[d0 + csz]) * 8],
                            num_idxs=nt * 128, num_idxs_reg=nt * 128, elem_size=128,
                            single_packet=False,
                            queue_num=qrr[0] % 4,
                        )
                        qrr[0] += 1
                        store[(ci, s)] = g

                    def build_oh_for(ci, s, store, g_store):
                        # one-hots are precomputed on the host and streamed in
                        # over HWDGE (DVE EQ builds would lock GpSimd out of
                        # SBUF and stall SWDGE descriptor generation)
                        d0, csz = chunks[ci]
                        offs = offa if s == "a" else offb
                        nt = int(offs[d0 + csz] - offs[d0])
                        oh2 = goa.tile([128, gmax, 256], bf16, tag="oh_s", name="oh2")
                        nc.sync.dma_start(
                            out=oh2[:, :nt, :].rearrange("p t e -> p (t e)"),
                            in_=ohp_in[s][:, int(offs[d0]) * 256:int(offs[d0 + csz]) * 256])
                        store[(ci, s)] = oh2

                    # phase 1: stream b -> partial; phase 2 (rotated order):
                    # stream a + combine + epilogue.  ECH puts tiles 24..48
                    # first so the b-half table collective for the next layer
                    # fires mid-phase (gated; full pipeline), and the a-half
                    # at layer end with a whole phase of slack.
                    slots = ([(ci, "b") for ci in range(nch)]
                             + [(ci, "a") for ci in range(12, nch)]
                             + [(ci, "a") for ci in range(12)])
                    g_store = {}
                    oh_store = {}
                    for si in range(min(LEAD, len(slots))):
                        issue_g(*slots[si], g_store)
                    for si in range(min(OHLEAD, len(slots))):
                        build_oh_for(*slots[si], oh_store, g_store)
                    pos = 0
                    for si, (ci, s) in enumerate(slots):
                        if si + LEAD < len(slots):
                            issue_g(*slots[si + LEAD], g_store)
                        if si + OHLEAD < len(slots):
                            build_oh_for(*slots[si + OHLEAD], oh_store, g_store)
                        d0, csz = chunks[ci]
                        g = g_store.pop((ci, s))
                        oh2 = oh_store.pop((ci, s))
                        if s == "b":
                            for i in range(csz):
                                t = d0 + i
                                ps_a = psA.tile([128, 128], f32, tag="ps_agg", name="ps_a")
                                nb_ = int(Bb[t])
                                nc.tensor.matmul(
                                    out=ps_a[:64, :], lhsT=strip[l % 2][:, t * 128:t * 128 + 64],
                                    rhs=ident_b[:], start=True, stop=(nb_ == 0))
                                base = int(offb[t] - offb[d0])
                                for j in range(nb_):
                                    nc.tensor.matmul(
                                        out=ps_a[:64, :], lhsT=g[:, base + j, 0:64],
                                        rhs=oh2[:, base + j, 0:128], start=False, stop=False)
                                    nc.tensor.matmul(
                                        out=ps_a[:64, :], lhsT=g[:, base + j, 64:128],
                                        rhs=oh2[:, base + j, 128:256], start=False,
                                        stop=(j == nb_ - 1))
                                nc.scalar.copy(out=partial[:64, t * 128:(t + 1) * 128],
                                               in_=ps_a[:64, :])
                        else:
                            for i in range(csz):
                                t = d0 + i
                                na_ = int(Ba[t])
                                t1 = wp.tile([128, 128], bf16, tag="t1", name="t1")
                                if na_ > 0:
                                    ps_b = psA.tile([128, 128], f32, tag="ps_agg", name="ps_b")
                                    base = int(offa[t] - offa[d0])
                                    for j in range(na_):
                                        nc.tensor.matmul(
                                            out=ps_b[:64, :], lhsT=g[:, base + j, 0:64],
                                            rhs=oh2[:, base + j, 0:128], start=(j == 0), stop=False)
                                        nc.tensor.matmul(
                                            out=ps_b[:64, :], lhsT=g[:, base + j, 64:128],
                                            rhs=oh2[:, base + j, 128:256], start=False,
                                            stop=(j == na_ - 1))
                                    nc.vector.tensor_tensor(
                                        out=t1[:64, :], in0=partial[:64, t * 128:(t + 1) * 128],
                                        in1=ps_b[:64, :], op=ADD)
                                else:
                                    nc.vector.tensor_copy(
                                        out=t1[:64, :], in_=partial[:64, t * 128:(t + 1) * 128])
                                epilogue(l, t, t1, pos)
                                pos += 1
                            if not last:
                                dma_rows(l + 1, d0, csz,
                                         strip[(l + 1) % 2][:, d0 * 128:(d0 + csz) * 128])
                                if d0 == NTILE - 1:
                                    cc_half(l + 1, 1)
                                    # pause gather issue until the collective
                                    # completes (head-of-line data dep)
                                    cc_gate(tf_b[l + 1], "b")
                    if not last:
                        cc_half(l + 1, 0)

            # ================= per-graph softmax tail =================
            with tc.tile_pool(name="tail", bufs=1) as tp:
                brow = tp.tile([128, NPAD], bf16, tag="brow", name="brow")
                nc.sync.dma_start(out=brow[:], in_=brow_in[:])
                sb0 = tp.tile([128, NTILE * 128], bf16, tag="sb0", name="sb0")
                sb1 = tp.tile([128, NTILE * 128], bf16, tag="sb1", name="sb1")

                gsum = pp.tile([128, 2], f32, tag="gsum", name="gsum")
                nc.scalar.copy(out=gsum[:, 0:1], in_=pg0[:])
                nc.scalar.copy(out=gsum[:, 1:2], in_=pg1[:])
                nc.sync.dma_start(out=ag_in[:], in_=gsum[:])
                nc.gpsimd.collective_compute(
                    "AllGather", mybir.AluOpType.bypass, replica_groups=rg,
                    ins=[ag_in[:]], outs=[ag_out[:]])
                # overlap the collective with the one-hot builds for the final pass
                for h_, sb in ((0, sb0), (1, sb1)):
                    nc.vector.tensor_scalar(sb[:], brow[:], vec[:, 9 + h_:10 + h_], None, EQ)
                s_all = pp.tile([128, 2 * NCORES], f32, tag="s_all", name="s_all")
                nc.sync.dma_start(
                    out=s_all[:].rearrange("p (r c) -> p r c", c=2),
                    in_=ag_out[:].rearrange("(r p) c -> p r c", p=128))
                s_red = pp.tile([128, 2], f32, tag="s_red", name="s_red")
                nc.vector.tensor_tensor(out=s_red[:], in0=s_all[:, 0:2], in1=s_all[:, 2:4], op=ADD)
                for r in range(2, NCORES):
                    nc.vector.tensor_tensor(out=s_red[:], in0=s_red[:],
                                            in1=s_all[:, 2 * r:2 * r + 2], op=ADD)
                r_all = pp.tile([128, 2], f32, tag="r_all", name="r_all")
                nc.vector.reciprocal(out=r_all[:], in_=s_red[:])
                r_b = pp.tile([128, 2], bf16, tag="r_b", name="r_b")
                nc.vector.tensor_copy(out=r_b[:], in_=r_all[:])
                for h_, sb in ((0, sb0), (1, sb1)):
                    for t in range(NTILE):
                        ps_r = psB.tile([128, 1], f32, tag="ps_tmp", name="ps_r")
                        nc.tensor.matmul(out=ps_r[:], lhsT=sb[:, t * 128:(t + 1) * 128],
                                         rhs=r_b[:, h_:h_ + 1], start=True, stop=True)
                        if h_ == 0:
                            nc.scalar.copy(out=r_str[:, t:t + 1], in_=ps_r[:])
                        else:
                            tmp = wp.tile([128, 1], f32, tag="tmp_r", name="tmp_r")
                            nc.vector.tensor_tensor(out=tmp[:], in0=r_str[:, t:t + 1],
                                                    in1=ps_r[:], op=ADD)
                            nc.vector.tensor_tensor(out=out_strip[:, t:t + 1],
                                                    in0=e_strip[:, t:t + 1], in1=tmp[:],
                                                    op=mybir.AluOpType.mult)
                nc.sync.dma_start(
                    out=out_dram[:].rearrange("(t p) one -> p (t one)", p=128),
                    in_=out_strip[:])

    nc.compile()
    return nc


def _in_maps(per_core, w):
    shared = dict(
        wlin=w["wlin"], vec=w["vec"], iota_e=w["iota_e"],
        iota_g0=w["iota_g0"], iota_g1=w["iota_g1"],
        ident_b=w["ident_b"],
    )
    for l in range(3):
        shared[f"w1_{l}"] = w[f"w1_{l}"]
        shared[f"w2_{l}"] = w[f"w2_{l}"]
    maps = []
    for r in range(NCORES):
        pc = per_core[r]
        maps.append(dict(
            x_bf=pc["x_bf"], x_edges=pc["x_edges"], drel_c=pc["drel_c"],
            idx_a=pc["idx_a"], idx_b=pc["idx_b"],
            ohp_a=pc["ohp_a"], ohp_b=pc["ohp_b"],
            brow=pc["brow"], batchT=pc["batchT"], **shared,
        ))
    return maps


def kernel(**inputs):
    import time
    from concourse.bass_utils import run_bass_kernel_spmd

    per_core, shape_key = _preprocess(inputs["x"], inputs["edge_index"], inputs["batch"])
    w = _weights(inputs)

    key = ("v20", shape_key)
    if key not in _CACHE:
        _CACHE[key] = _build(shape_key, w["blin_t"])
    nc = _CACHE[key]

    maps = _in_maps(per_core, w)
    last = None
    for attempt in range(3):
        try:
            res = run_bass_kernel_spmd(nc, maps, list(range(NCORES)))
            break
        except Exception as e:   # wedged device from a prior crash: retry
            last = e
            time.sleep(20)
    else:
        raise last
    out = np.concatenate([res.results[r]["out"][:NLOC] for r in range(NCORES)], axis=0)
    return out.astype(np.float32)
